# revision 7
# baseline (speedup 1.0000x reference)
"""Trainium2 Bass kernel for nn_Loss_343597383760.

Loss:
    scores = predicted_values[rel_idx, e1_idx, e2_idx]        # [N] gather
    sig    = sigmoid(scores)
    total  = sum(lab*sig + (1-lab)*(1-sig)) = neg + sum(w*sig),  w = 2*lab-1
    loss   = -total / ((1+neg)*N)

Sharding (expert-style, per relation): core c owns relations {2c, 2c+1} of
predicted_values ([2,4096,4096] f32 = 128 MiB per core). Host buckets the
262144 triplets by owning core and converts each to a flat element index
into the local shard.

Device-side weight multiply is eliminated by layout: the host packs the
core's positive-label triplets into columns [0, BPOS) of the [128, COLS]
index plane and the negative-label ones into [BPOS, COLS), padding each
region with index 0. The device gathers each region (indirect SWDGE DMA)
and computes per-region sigmoid sums with ACT accum_out — no weights, no
DVE pass. The host recovers
    sum w*sig(s) = (S_pos - npad_pos*sig0_c) - (S_neg - npad_neg*sig0_c)
with sig0_c = sigmoid(pv_shard_c[0]) (the value pad slots gather).

Chunking: one indirect gather per region (the Pool sequencer cost of an
indirect DMA is ~1.1us regardless of size, so fewer chunks win); the
region-0 index plane ships via a dedicated early DMA on the SP queue so
the first gather dispatches as soon as its completion sem fires; each
region's [128,1] partial sum is DMAed out right after its accumulate so
the completion receipts overlap the remaining work.
"""

import numpy as np

import concourse.bass as bass
import concourse.bacc as bacc
import concourse.tile as tile
from concourse import mybir
from concourse.bass_utils import run_bass_kernel_spmd

R, E, N = 16, 4096, 262144
NCORES = 8
RPC = R // NCORES            # relations per core
TOTAL = RPC * E * E          # elements in one core's shard
P = 128                      # SBUF partitions

BPOS = 132                   # columns reserved for positive-label triplets
BNEG = 132                   # columns for negative-label triplets
COLS = BPOS + BNEG           # 264 -> capacity 33792 (max bucket ~33040)
CAPP = P * BPOS              # per-region slot capacity (16896; region count ~16520)
CAP = P * COLS
NQ = 2                       # SWDGE queues; the two gathers use different rings

# Set by test harness to capture a neuron-profile trace.
TRACE = False
LAST_RESULTS = None

_NC = None


def _indirect_gather_q(nc, out, in_, in_offset, queue_name):
    """indirect_dma_start with an explicit SWDGE queue (the stock API pins
    qPoolDynamic; two queues let the second gather's ring drain overlap)."""
    orig = mybir.InstDMACopy

    def patched(**kw):
        kw["queue"] = queue_name
        return orig(**kw)

    mybir.InstDMACopy = patched
    try:
        return nc.gpsimd.indirect_dma_start(
            out=out, out_offset=None, in_=in_, in_offset=in_offset
        )
    finally:
        mybir.InstDMACopy = orig


def _build_nc():
    f32 = mybir.dt.float32
    i32 = mybir.dt.int32
    nc = bacc.Bacc(num_swdge_queues=NQ)
    pv = nc.declare_dram_parameter("pv", [TOTAL, 1], f32, isOutput=False)
    idxs = nc.declare_dram_parameter("idx", [P, COLS], i32, isOutput=False)
    out = nc.declare_dram_parameter("out", [P, 2], f32, isOutput=True)

    with (
        tile.TileContext(nc) as tc,
        tc.tile_pool(name="io", bufs=1) as io_pool,
        tc.tile_pool(name="work", bufs=1) as work_pool,
        tc.tile_pool(name="res", bufs=1) as res_pool,
    ):
        outbuf = res_pool.tile([P, 2], f32)
        it0 = io_pool.tile([P, BPOS], i32, tag="idx0")
        nc.sync.dma_start(out=it0[:], in_=idxs[:, :BPOS])
        it1 = io_pool.tile([P, BNEG], i32, tag="idx1")
        nc.scalar.dma_start(out=it1[:], in_=idxs[:, BPOS:])
        for k, (it, c) in enumerate(((it0, BPOS), (it1, BNEG))):
            g = work_pool.tile([P, c], f32, tag=f"gath{k}")
            _indirect_gather_q(
                nc,
                out=g[:],
                in_=pv[:],
                in_offset=bass.IndirectOffsetOnAxis(ap=it[:], axis=0),
                queue_name=f"qPoolDynamic{k % NQ or ''}",
            )
            sg = work_pool.tile([P, c], f32, tag=f"sig{k}")
            nc.scalar.activation(
                out=sg[:],
                in_=g[:],
                func=mybir.ActivationFunctionType.Sigmoid,
                accum_out=outbuf[:, k : k + 1],
            )
            # per-region result DMA so the completion receipt of region 0
            # overlaps region 1's gather/sigmoid instead of the kernel tail
            eng = nc.sync if k == 0 else nc.scalar
            eng.dma_start(out=out[:, k : k + 1], in_=outbuf[:, k : k + 1])
    nc.finalize()
    return nc


def kernel(predicted_values, rel_idx, e1_idx, e2_idx, labels):
    global _NC, LAST_RESULTS
    pv = np.ascontiguousarray(np.asarray(predicted_values, dtype=np.float32))
    rel = np.asarray(rel_idx, dtype=np.int64)
    e1 = np.asarray(e1_idx, dtype=np.int64)
    e2 = np.asarray(e2_idx, dtype=np.int64)
    lab = np.asarray(labels, dtype=np.int64)

    owner = rel // RPC
    local_flat = (rel % RPC) * (E * E) + e1 * E + e2  # < TOTAL, fits int32
    pos = lab == 1

    pv_flat = pv.reshape(R * E * E)
    host_extra = 0.0   # sum of w*sig for overflow triplets (host-computed)
    pad_corr = 0.0     # sum over cores of (npad_pos - npad_neg) * sig0_c
    in_maps = []
    for c in range(NCORES):
        m = owner == c
        fp = local_flat[m & pos]
        fn = local_flat[m & ~pos]
        # overflow beyond a region's capacity: host computes those terms
        for arr, w in ((fp[CAPP:], 1.0), (fn[CAPP:], -1.0)):
            if arr.size:
                s = pv_flat[arr + c * TOTAL].astype(np.float64)
                host_extra += w * float(np.sum(1.0 / (1.0 + np.exp(-s))))
        fp = fp[:CAPP]
        fn = fn[:CAPP]
        sig0 = 1.0 / (1.0 + np.exp(-float(pv_flat[c * TOTAL])))
        pad_corr += ((CAPP - fp.size) - (CAPP - fn.size)) * sig0
        plane = np.zeros((P, COLS), np.int32)
        p_arr = np.zeros(CAPP, np.int32)
        p_arr[: fp.size] = fp.astype(np.int32)
        n_arr = np.zeros(CAPP, np.int32)
        n_arr[: fn.size] = fn.astype(np.int32)
        plane[:, :BPOS] = p_arr.reshape(P, BPOS)
        plane[:, BPOS:] = n_arr.reshape(P, BNEG)
        in_maps.append(
            {
                "pv": pv[c * RPC : (c + 1) * RPC].reshape(TOTAL, 1),
                "idx": plane,
            }
        )

    if _NC is None:
        _NC = _build_nc()

    res = run_bass_kernel_spmd(
        _NC, in_maps, core_ids=list(range(NCORES)), trace=TRACE
    )
    LAST_RESULTS = res

    # device: out[:,0] = per-partition sum sig over region 0 (positives+pads),
    #         out[:,1] = same over region 1 (negatives+pads)
    asig = host_extra - pad_corr
    for c in range(NCORES):
        o = np.asarray(res.results[c]["out"], dtype=np.float64)
        asig += float(o[:, 0].sum()) - float(o[:, 1].sum())

    neg = float(np.sum(lab == 0))
    loss = -(neg + asig) / ((1.0 + neg) * float(N))
    return np.array([loss], dtype=np.float32)
